# revision 22
# baseline (speedup 1.0000x reference)
"""BitLinear (int4-fakequant x @ ternary-weight linear) Trainium2 Bass kernel.

Math (per reference):
  maxabs[s] = max(|x[s, :]|) clamped to >= 1e-6
  q[s, k]   = round(x[s, k] / maxabs[s] * 7)           # in [-7, 7]
  xq        = q * maxabs / 7
  thresh    = 0.05 * mean(|w|)                          # global scalar
  sign[o,k] = 0 if |w[o,k]| < thresh else sign(w[o,k])  # in {-1, 0, 1}
  alpha[o]  = mean(|w[o, :]|)
  out[s, o] = (maxabs[s]/7) * alpha[o] * S[s,o] + bias[o],  S = q @ sign.T

S is an exact small-integer matmul computed on the PE array in fp8 e4m3
(ints -7..7 exact; fp32 PSUM accumulation, |S| <= 28672 < 2^24, so S is
EXACT). Host precomputes the sharding layout: q and sign are shipped as
fp8 codes pre-tiled to the SBUF layout (each DMA is per-partition
contiguous, ~128 descriptors), so the PE ramp is DMA-bandwidth bound
only. The O(N*K*O) matmul plus all output scaling run on device.
Column-parallel over out_f across 8 cores (sign/alpha/bias/out sharded,
q replicated).

Device per-core schedule (M=8192 rows, K=4096, O_SH=2048):
  Two phases over o-halves: phase A computes o-tiles 0-1 for all
  s-blocks, phase B o-tiles 2-3. Only half the sign bytes (4.2 MB) must
  land before the sweep streams; the other halves + alpha load under
  phase-A compute. q8 is re-streamed per phase (HBM has 2x headroom
  over the PE here). Per 128-row s-block: 16 DoubleRow matmuls x 2 psum
  o-tiles (FD=512, 216 ns/MM = the fp8 silicon floor), PSUM evicted on
  ACT with per-partition scale rs[s]=maxabs/7, DVE multiply by alpha[o]
  broadcast (+bias), DMA out per o-tile.
"""

import numpy as np

import concourse.bacc as bacc
import concourse.bass as bass
import concourse.mybir as mybir
import concourse.tile as tile
from concourse.bass import ts

F32 = mybir.dt.float32
FP8 = mybir.dt.float8e4
AOP = mybir.AluOpType

P = 128
OTILE = 512          # psum free-dim tile (one bank of fp32)
CHB = 8              # s-blocks per q8 DMA chunk
NO_PH = 2            # o-tiles per phase


def build_nc(M, IN_F, O_SH, with_bias):
    """Build the per-core SPMD program. Shapes are per-core shard shapes."""
    KSUB = IN_F // P          # k-subtiles (must be even for DoubleRow pairs)
    NBLK = M // P             # s-blocks of 128 rows
    NOT = O_SH // OTILE       # psum o-tiles
    NPAIR = KSUB // 2
    NCH = NBLK // CHB         # q8 chunks
    CHW = KSUB * CHB * P      # q8 chunk bytes per partition
    OPH = NO_PH * OTILE       # o-columns per phase
    NPH = NOT // NO_PH
    SW = NPAIR * 2 * OPH      # sign bytes per partition per phase
    assert KSUB % 2 == 0 and M % (P * CHB) == 0 and NOT % NO_PH == 0

    nc = bacc.Bacc("TRN2", target_bir_lowering=False, debug=False)

    # q8t: chunk-major pre-tiled int4 codes of x; row c*P+p holds, for ko in
    # 0..KSUB, the 128*CHB s-columns of chunk c from k-row ko*P+p.
    q8t = nc.dram_tensor("q8t", [NCH * P, CHW], FP8, kind="ExternalInput").ap()
    # s8t: phase-major pre-tiled ternary sign; row ph*P+p holds, for kk, pair,
    # the OPH o-columns of phase ph from k-row (2*kk+pair)*P+p.
    s8t = nc.dram_tensor("s8t", [NPH * P, SW], FP8, kind="ExternalInput").ap()
    rs = nc.dram_tensor("rs", [P, NBLK], F32, kind="ExternalInput").ap()
    alpha = nc.dram_tensor("alpha", [1, O_SH], F32, kind="ExternalInput").ap()
    if with_bias:
        bias = nc.dram_tensor("bias", [1, O_SH], F32, kind="ExternalInput").ap()
    out = nc.dram_tensor("out", [M, O_SH], F32, kind="ExternalOutput").ap()

    q8_r = q8t.rearrange("(c p) (ko w) -> c p ko w", p=P, ko=KSUB)
    s8_r = s8t.rearrange("(f p) (kk pr o) -> f p kk pr o", p=P, kk=NPAIR, pr=2)
    out_r = out.rearrange("(t p) o -> p t o", p=P)    # [128, NBLK, O_SH]

    with tile.TileContext(nc) as tc:
        with (
            tc.tile_pool(name="const", bufs=1) as constp,
            tc.tile_pool(name="sign", bufs=1) as signp,
            tc.tile_pool(name="q8p", bufs=3) as q8p,
            tc.tile_pool(name="outp", bufs=4) as outp,
            tc.tile_pool(name="psum", bufs=8, space="PSUM") as psum,
        ):
            sign_tiles = [
                signp.tile([P, NPAIR, 2, OPH], FP8, tag=f"sg{f}", name=f"sg{f}")
                for f in range(NPH)
            ]

            def load_chunk(ph, c, nsplit=1):
                qt = q8p.tile([P, KSUB, CHB * P], FP8, tag="q8", name=f"q8_{ph}_{c}")
                js = KSUB // nsplit
                for j in range(nsplit):
                    nc.sync.dma_start(
                        qt[:, j * js : (j + 1) * js, :],
                        q8_r[c, :, j * js : (j + 1) * js, :],
                    )
                return qt

            # PE warm-up: junk matmuls on uninitialized SBUF (values never
            # read; the psum slot is overwritten by the first real start=True
            # group). No input deps, so these queue at Tensor-engine init
            # (~+4.4us) and the HAM clock gate is at 8/8 (2.4 GHz) by the
            # first real matmul at ~+11us.
            junk = constp.tile([P, 2, OTILE], FP8, tag="junk")
            nc.vector.memset(junk[:], 0.0)
            ps_warm = psum.tile([P, OTILE], F32, tag="ps", name="ps_warm")
            for _ in range(8):
                nc.tensor.matmul(
                    ps_warm[:],
                    junk[:, :, 0:P],
                    junk[:],
                    start=True,
                    stop=True,
                    perf_mode=mybir.MatmulPerfMode.DoubleRow,
                )

            # Startup: q8 chunk-0 eighths issue on the Sync queue while the
            # phase-A sign eighths issue in parallel on the Scalar queue
            # (descriptor-gen is ~0.6us per dma_start and serializes per
            # queue; two queues halve the issue ramp). alpha + phase-B signs
            # follow on Scalar, chunks + outputs own Sync.
            qt_cur = q8p.tile([P, KSUB, CHB * P], FP8, tag="q8", name="q8_0_0")
            for j in range(8):
                jk = KSUB // 8
                nc.sync.dma_start(
                    qt_cur[:, j * jk : (j + 1) * jk, :],
                    q8_r[0, :, j * jk : (j + 1) * jk, :],
                )
                jq = NPAIR // 8
                nc.scalar.dma_start(
                    sign_tiles[0][:, j * jq : (j + 1) * jq, :, :],
                    s8_r[0, :, j * jq : (j + 1) * jq, :, :],
                )
            rs_sb = constp.tile([P, NBLK], F32, tag="rs_sb")
            nc.scalar.dma_start(rs_sb[:], rs[:, :])
            qt_next = load_chunk(0, 1, nsplit=2)
            alpha_bc = constp.tile([P, O_SH], F32, tag="alpha_bc")
            nc.scalar.dma_start(alpha_bc[:], alpha[0:1, :].to_broadcast((P, O_SH)))
            if with_bias:
                bias_bc = constp.tile([P, O_SH], F32, tag="bias_bc")
                nc.scalar.dma_start(bias_bc[:], bias[0:1, :].to_broadcast((P, O_SH)))
            for f in range(1, NPH):
                for j in range(2):
                    hk = NPAIR // 2
                    nc.scalar.dma_start(
                        sign_tiles[f][:, j * hk : (j + 1) * hk, :, :],
                        s8_r[f, :, j * hk : (j + 1) * hk, :, :],
                    )

            chunks = [(ph, c) for ph in range(NPH) for c in range(NBLK // CHB)]

            for gi, (ph, c) in enumerate(chunks):
                ob = ph * NO_PH
                sg = sign_tiles[ph]
                if gi > 0:
                    qt_cur = qt_next
                    if gi + 1 < len(chunks):
                        qt_next = load_chunk(*chunks[gi + 1])
                for r in range(CHB):
                    t = c * CHB + r
                    out_t = outp.tile([P, OPH], F32, tag="out")
                    ps_tiles = [
                        psum.tile([P, OTILE], F32, tag="ps", name=f"ps_{ph}_{t}_{i}")
                        for i in range(NO_PH)
                    ]
                    for kk in range(NPAIR):
                        lhsT = qt_cur[:, 2 * kk : 2 * kk + 2, ts(r, P)]
                        for oi in range(NO_PH):
                            nc.tensor.matmul(
                                ps_tiles[oi][:],
                                lhsT,
                                sg[:, kk, :, ts(oi, OTILE)],
                                start=(kk == 0),
                                stop=(kk == NPAIR - 1),
                                perf_mode=mybir.MatmulPerfMode.DoubleRow,
                            )
                    for oi in range(NO_PH):
                        # rowscale applied on PSUM eviction (per-partition)
                        nc.scalar.activation(
                            out_t[:, ts(oi, OTILE)],
                            ps_tiles[oi][:],
                            mybir.ActivationFunctionType.Copy,
                            scale=rs_sb[:, t : t + 1],
                        )
                        o0 = (ob + oi) * OTILE
                        nc.vector.tensor_tensor(
                            out_t[:, ts(oi, OTILE)],
                            out_t[:, ts(oi, OTILE)],
                            alpha_bc[:, o0 : o0 + OTILE],
                            AOP.mult,
                        )
                        if with_bias:
                            nc.vector.tensor_tensor(
                                out_t[:, ts(oi, OTILE)],
                                out_t[:, ts(oi, OTILE)],
                                bias_bc[:, o0 : o0 + OTILE],
                                AOP.add,
                            )
                        nc.sync.dma_start(
                            out_r[:, t, o0 : o0 + OTILE], out_t[:, ts(oi, OTILE)]
                        )

    nc.compile()
    return nc


# e4m3 (bias 7) byte codes for integers -7..7; index by q+7.
_E4M3_INT = np.array(
    [0xCE, 0xCC, 0xCA, 0xC8, 0xC4, 0xC0, 0xB8, 0x00,
     0x38, 0x40, 0x44, 0x48, 0x4A, 0x4C, 0x4E],
    dtype=np.uint8,
)


def host_prep(x, weight, bias, n_cores):
    """Host-side quantize + pre-tiled layout prep. Returns per-core maps."""
    import ml_dtypes

    IN_F = x.shape[-1]
    OUT_F = weight.shape[0]
    M = int(np.prod(x.shape[:-1]))
    O_SH = OUT_F // n_cores
    NBLK = M // P
    KSUB = IN_F // P
    NPAIR = KSUB // 2
    NCH = NBLK // CHB
    NOT = O_SH // OTILE
    OPH = NO_PH * OTILE
    NPH = NOT // NO_PH

    x2 = x.reshape(M, IN_F)
    maxabs = np.maximum(np.abs(x2).max(axis=1), 1e-6).astype(np.float32)
    rs = (maxabs / np.float32(7.0)).astype(np.float32)
    rs_striped = np.ascontiguousarray(rs.reshape(NBLK, P).T)  # [128, NBLK]

    # int4 codes of x, pre-tiled chunk-major: [NCH*P, KSUB*CHB*P] where row
    # c*P+p = q[k=ko*P+p, s=c*CHB*P + 0..CHB*P) for ko ascending.
    qi = np.rint(x2 * (np.float32(7.0) / maxabs)[:, None]).astype(np.int8)
    q8 = _E4M3_INT[(qi + 7).astype(np.uint8)]          # [M, IN_F] codes
    q8t = (
        q8.T.reshape(KSUB, P, NCH, CHB * P)
        .transpose(2, 1, 0, 3)
        .reshape(NCH * P, KSUB * CHB * P)
    )
    q8t = np.ascontiguousarray(q8t).view(ml_dtypes.float8_e4m3)

    thresh = np.float32(0.05) * np.float32(np.abs(weight).mean(dtype=np.float64))
    with_bias = bool(np.any(bias))

    in_maps = []
    for c in range(n_cores):
        o0, o1 = c * O_SH, (c + 1) * O_SH
        w_sh = weight[o0:o1]
        # ternary sign codes {0x00, 0x38, 0xB8}, pre-tiled phase-major:
        # [NPH*P, NPAIR*2*OPH], row f*P+p = sign[k=(2kk+pr)*P+p, o-phase f]
        si = np.where(np.abs(w_sh) < thresh, np.int8(0), np.sign(w_sh).astype(np.int8))
        s8 = _E4M3_INT[(si + 7).astype(np.uint8)]       # [O_SH, IN_F] codes
        s8t = (
            s8.T.reshape(KSUB, P, NPH, OPH)
            .transpose(2, 1, 0, 3)
            .reshape(NPH * P, KSUB * OPH)
        )
        s8t = np.ascontiguousarray(s8t).view(ml_dtypes.float8_e4m3)
        m = {
            "q8t": q8t,
            "s8t": s8t,
            "rs": rs_striped,
            "alpha": np.abs(w_sh).mean(axis=1, dtype=np.float32).reshape(1, O_SH),
        }
        if with_bias:
            m["bias"] = np.ascontiguousarray(bias[o0:o1], dtype=np.float32).reshape(
                1, O_SH
            )
        in_maps.append(m)
    return in_maps, with_bias


_NC_CACHE = {}


def _get_nc(M, IN_F, O_SH, with_bias):
    key = (M, IN_F, O_SH, with_bias)
    if key not in _NC_CACHE:
        _NC_CACHE[key] = build_nc(M, IN_F, O_SH, with_bias)
    return _NC_CACHE[key]


def kernel(x, weight, bias, _trace=False):
    from concourse.bass_utils import run_bass_kernel_spmd

    N_CORES = 8
    x = np.asarray(x)
    weight = np.asarray(weight)
    bias = np.asarray(bias)
    IN_F = x.shape[-1]
    OUT_F = weight.shape[0]
    M = int(np.prod(x.shape[:-1]))
    O_SH = OUT_F // N_CORES

    in_maps, with_bias = host_prep(x, weight, bias, N_CORES)
    nc = _get_nc(M, IN_F, O_SH, with_bias)
    res = run_bass_kernel_spmd(
        nc, in_maps, core_ids=list(range(N_CORES)), trace=_trace
    )
    parts = [res.results[c]["out"].reshape(*x.shape[:-1], O_SH) for c in range(N_CORES)]
    full = np.concatenate(parts, axis=-1)
    if with_bias is False and np.any(bias):  # pragma: no cover (safety)
        full = full + bias
    if _trace:
        return full, res
    return full


# revision 23
# speedup vs baseline: 1.0018x; 1.0018x over previous
"""BitLinear (int4-fakequant x @ ternary-weight linear) Trainium2 Bass kernel.

Math (per reference):
  maxabs[s] = max(|x[s, :]|) clamped to >= 1e-6
  q[s, k]   = round(x[s, k] / maxabs[s] * 7)           # in [-7, 7]
  xq        = q * maxabs / 7
  thresh    = 0.05 * mean(|w|)                          # global scalar
  sign[o,k] = 0 if |w[o,k]| < thresh else sign(w[o,k])  # in {-1, 0, 1}
  alpha[o]  = mean(|w[o, :]|)
  out[s, o] = (maxabs[s]/7) * alpha[o] * S[s,o] + bias[o],  S = q @ sign.T

S is an exact small-integer matmul computed on the PE array in fp8 e4m3
(ints -7..7 exact; fp32 PSUM accumulation, |S| <= 28672 < 2^24, so S is
EXACT). Host precomputes the sharding layout: q and sign are shipped as
fp8 codes pre-tiled to the SBUF layout (each DMA is per-partition
contiguous, ~128 descriptors), so the PE ramp is DMA-bandwidth bound
only. The O(N*K*O) matmul plus all output scaling run on device.
Column-parallel over out_f across 8 cores (sign/alpha/bias/out sharded,
q replicated).

Device per-core schedule (M=8192 rows, K=4096, O_SH=2048):
  Two phases over o-halves: phase A computes o-tiles 0-1 for all
  s-blocks, phase B o-tiles 2-3. Only half the sign bytes (4.2 MB) must
  land before the sweep streams; the other halves + alpha load under
  phase-A compute. q8 is re-streamed per phase (HBM has 2x headroom
  over the PE here). Per 128-row s-block: 16 DoubleRow matmuls x 2 psum
  o-tiles (FD=512, 216 ns/MM = the fp8 silicon floor), PSUM evicted on
  ACT with per-partition scale rs[s]=maxabs/7, DVE multiply by alpha[o]
  broadcast (+bias), DMA out per o-tile.
"""

import numpy as np

import concourse.bacc as bacc
import concourse.bass as bass
import concourse.mybir as mybir
import concourse.tile as tile
from concourse.bass import ts

F32 = mybir.dt.float32
FP8 = mybir.dt.float8e4
AOP = mybir.AluOpType

P = 128
OTILE = 512          # psum free-dim tile (one bank of fp32)
CHB = 8              # s-blocks per q8 DMA chunk
NO_PH = 2            # o-tiles per phase


def build_nc(M, IN_F, O_SH, with_bias):
    """Build the per-core SPMD program. Shapes are per-core shard shapes."""
    KSUB = IN_F // P          # k-subtiles (must be even for DoubleRow pairs)
    NBLK = M // P             # s-blocks of 128 rows
    NOT = O_SH // OTILE       # psum o-tiles
    NPAIR = KSUB // 2
    NCH = NBLK // CHB         # q8 chunks
    CHW = KSUB * CHB * P      # q8 chunk bytes per partition
    OPH = NO_PH * OTILE       # o-columns per phase
    NPH = NOT // NO_PH
    SW = NPAIR * 2 * OPH      # sign bytes per partition per phase
    assert KSUB % 2 == 0 and M % (P * CHB) == 0 and NOT % NO_PH == 0

    nc = bacc.Bacc("TRN2", target_bir_lowering=False, debug=False)

    # q8t: chunk-major pre-tiled int4 codes of x; row c*P+p holds, for ko in
    # 0..KSUB, the 128*CHB s-columns of chunk c from k-row ko*P+p.
    q8t = nc.dram_tensor("q8t", [NCH * P, CHW], FP8, kind="ExternalInput").ap()
    # s8t: phase-major pre-tiled ternary sign; row ph*P+p holds, for kk, pair,
    # the OPH o-columns of phase ph from k-row (2*kk+pair)*P+p.
    s8t = nc.dram_tensor("s8t", [NPH * P, SW], FP8, kind="ExternalInput").ap()
    rs = nc.dram_tensor("rs", [P, NBLK], F32, kind="ExternalInput").ap()
    alpha = nc.dram_tensor("alpha", [1, O_SH], F32, kind="ExternalInput").ap()
    if with_bias:
        bias = nc.dram_tensor("bias", [1, O_SH], F32, kind="ExternalInput").ap()
    out = nc.dram_tensor("out", [M, O_SH], F32, kind="ExternalOutput").ap()

    q8_r = q8t.rearrange("(c p) (ko w) -> c p ko w", p=P, ko=KSUB)
    s8_r = s8t.rearrange("(f p) (kk pr o) -> f p kk pr o", p=P, kk=NPAIR, pr=2)
    out_r = out.rearrange("(t p) o -> p t o", p=P)    # [128, NBLK, O_SH]

    with tile.TileContext(nc) as tc:
        with (
            tc.tile_pool(name="const", bufs=1) as constp,
            tc.tile_pool(name="sign", bufs=1) as signp,
            tc.tile_pool(name="q8p", bufs=3) as q8p,
            tc.tile_pool(name="outp", bufs=4) as outp,
            tc.tile_pool(name="psum", bufs=8, space="PSUM") as psum,
        ):
            sign_tiles = [
                signp.tile([P, NPAIR, 2, OPH], FP8, tag=f"sg{f}", name=f"sg{f}")
                for f in range(NPH)
            ]

            def load_chunk(ph, c, nsplit=1):
                qt = q8p.tile([P, KSUB, CHB * P], FP8, tag="q8", name=f"q8_{ph}_{c}")
                js = KSUB // nsplit
                for j in range(nsplit):
                    nc.sync.dma_start(
                        qt[:, j * js : (j + 1) * js, :],
                        q8_r[c, :, j * js : (j + 1) * js, :],
                    )
                return qt

            # PE warm-up: junk matmuls on uninitialized SBUF (values never
            # read; the psum slot is overwritten by the first real start=True
            # group). No input deps, so these queue at Tensor-engine init
            # (~+4.4us) and the HAM clock gate is at 8/8 (2.4 GHz) by the
            # first real matmul at ~+11us.
            junk = constp.tile([P, 2, OTILE], FP8, tag="junk")
            nc.vector.memset(junk[:], 0.0)
            ps_warm = psum.tile([P, OTILE], F32, tag="ps", name="ps_warm")
            for _ in range(16):
                nc.tensor.matmul(
                    ps_warm[:],
                    junk[:, :, 0:P],
                    junk[:],
                    start=True,
                    stop=True,
                    perf_mode=mybir.MatmulPerfMode.DoubleRow,
                )

            # Startup: q8 chunk-0 eighths issue on the Sync queue while the
            # phase-A sign eighths issue in parallel on the Scalar queue
            # (descriptor-gen is ~0.6us per dma_start and serializes per
            # queue; two queues halve the issue ramp). alpha + phase-B signs
            # follow on Scalar, chunks + outputs own Sync.
            qt_cur = q8p.tile([P, KSUB, CHB * P], FP8, tag="q8", name="q8_0_0")
            for j in range(8):
                jk = KSUB // 8
                nc.sync.dma_start(
                    qt_cur[:, j * jk : (j + 1) * jk, :],
                    q8_r[0, :, j * jk : (j + 1) * jk, :],
                )
                jq = NPAIR // 8
                nc.scalar.dma_start(
                    sign_tiles[0][:, j * jq : (j + 1) * jq, :, :],
                    s8_r[0, :, j * jq : (j + 1) * jq, :, :],
                )
            rs_sb = constp.tile([P, NBLK], F32, tag="rs_sb")
            nc.scalar.dma_start(rs_sb[:], rs[:, :])
            qt_next = load_chunk(0, 1, nsplit=2)
            alpha_bc = constp.tile([P, O_SH], F32, tag="alpha_bc")
            nc.scalar.dma_start(alpha_bc[:], alpha[0:1, :].to_broadcast((P, O_SH)))
            if with_bias:
                bias_bc = constp.tile([P, O_SH], F32, tag="bias_bc")
                nc.scalar.dma_start(bias_bc[:], bias[0:1, :].to_broadcast((P, O_SH)))
            for f in range(1, NPH):
                for j in range(2):
                    hk = NPAIR // 2
                    nc.scalar.dma_start(
                        sign_tiles[f][:, j * hk : (j + 1) * hk, :, :],
                        s8_r[f, :, j * hk : (j + 1) * hk, :, :],
                    )

            chunks = [(ph, c) for ph in range(NPH) for c in range(NBLK // CHB)]

            for gi, (ph, c) in enumerate(chunks):
                ob = ph * NO_PH
                sg = sign_tiles[ph]
                if gi > 0:
                    qt_cur = qt_next
                    if gi + 1 < len(chunks):
                        qt_next = load_chunk(*chunks[gi + 1])
                for r in range(CHB):
                    t = c * CHB + r
                    out_t = outp.tile([P, OPH], F32, tag="out")
                    ps_tiles = [
                        psum.tile([P, OTILE], F32, tag="ps", name=f"ps_{ph}_{t}_{i}")
                        for i in range(NO_PH)
                    ]
                    for kk in range(NPAIR):
                        lhsT = qt_cur[:, 2 * kk : 2 * kk + 2, ts(r, P)]
                        for oi in range(NO_PH):
                            nc.tensor.matmul(
                                ps_tiles[oi][:],
                                lhsT,
                                sg[:, kk, :, ts(oi, OTILE)],
                                start=(kk == 0),
                                stop=(kk == NPAIR - 1),
                                perf_mode=mybir.MatmulPerfMode.DoubleRow,
                            )
                    for oi in range(NO_PH):
                        # rowscale applied on PSUM eviction (per-partition)
                        nc.scalar.activation(
                            out_t[:, ts(oi, OTILE)],
                            ps_tiles[oi][:],
                            mybir.ActivationFunctionType.Copy,
                            scale=rs_sb[:, t : t + 1],
                        )
                        o0 = (ob + oi) * OTILE
                        nc.vector.tensor_tensor(
                            out_t[:, ts(oi, OTILE)],
                            out_t[:, ts(oi, OTILE)],
                            alpha_bc[:, o0 : o0 + OTILE],
                            AOP.mult,
                        )
                        if with_bias:
                            nc.vector.tensor_tensor(
                                out_t[:, ts(oi, OTILE)],
                                out_t[:, ts(oi, OTILE)],
                                bias_bc[:, o0 : o0 + OTILE],
                                AOP.add,
                            )
                        nc.sync.dma_start(
                            out_r[:, t, o0 : o0 + OTILE], out_t[:, ts(oi, OTILE)]
                        )

    nc.compile()
    return nc


# e4m3 (bias 7) byte codes for integers -7..7; index by q+7.
_E4M3_INT = np.array(
    [0xCE, 0xCC, 0xCA, 0xC8, 0xC4, 0xC0, 0xB8, 0x00,
     0x38, 0x40, 0x44, 0x48, 0x4A, 0x4C, 0x4E],
    dtype=np.uint8,
)


def host_prep(x, weight, bias, n_cores):
    """Host-side quantize + pre-tiled layout prep. Returns per-core maps."""
    import ml_dtypes

    IN_F = x.shape[-1]
    OUT_F = weight.shape[0]
    M = int(np.prod(x.shape[:-1]))
    O_SH = OUT_F // n_cores
    NBLK = M // P
    KSUB = IN_F // P
    NPAIR = KSUB // 2
    NCH = NBLK // CHB
    NOT = O_SH // OTILE
    OPH = NO_PH * OTILE
    NPH = NOT // NO_PH

    x2 = x.reshape(M, IN_F)
    maxabs = np.maximum(np.abs(x2).max(axis=1), 1e-6).astype(np.float32)
    rs = (maxabs / np.float32(7.0)).astype(np.float32)
    rs_striped = np.ascontiguousarray(rs.reshape(NBLK, P).T)  # [128, NBLK]

    # int4 codes of x, pre-tiled chunk-major: [NCH*P, KSUB*CHB*P] where row
    # c*P+p = q[k=ko*P+p, s=c*CHB*P + 0..CHB*P) for ko ascending.
    qi = np.rint(x2 * (np.float32(7.0) / maxabs)[:, None]).astype(np.int8)
    q8 = _E4M3_INT[(qi + 7).astype(np.uint8)]          # [M, IN_F] codes
    q8t = (
        q8.T.reshape(KSUB, P, NCH, CHB * P)
        .transpose(2, 1, 0, 3)
        .reshape(NCH * P, KSUB * CHB * P)
    )
    q8t = np.ascontiguousarray(q8t).view(ml_dtypes.float8_e4m3)

    thresh = np.float32(0.05) * np.float32(np.abs(weight).mean(dtype=np.float64))
    with_bias = bool(np.any(bias))

    in_maps = []
    for c in range(n_cores):
        o0, o1 = c * O_SH, (c + 1) * O_SH
        w_sh = weight[o0:o1]
        # ternary sign codes {0x00, 0x38, 0xB8}, pre-tiled phase-major:
        # [NPH*P, NPAIR*2*OPH], row f*P+p = sign[k=(2kk+pr)*P+p, o-phase f]
        si = np.where(np.abs(w_sh) < thresh, np.int8(0), np.sign(w_sh).astype(np.int8))
        s8 = _E4M3_INT[(si + 7).astype(np.uint8)]       # [O_SH, IN_F] codes
        s8t = (
            s8.T.reshape(KSUB, P, NPH, OPH)
            .transpose(2, 1, 0, 3)
            .reshape(NPH * P, KSUB * OPH)
        )
        s8t = np.ascontiguousarray(s8t).view(ml_dtypes.float8_e4m3)
        m = {
            "q8t": q8t,
            "s8t": s8t,
            "rs": rs_striped,
            "alpha": np.abs(w_sh).mean(axis=1, dtype=np.float32).reshape(1, O_SH),
        }
        if with_bias:
            m["bias"] = np.ascontiguousarray(bias[o0:o1], dtype=np.float32).reshape(
                1, O_SH
            )
        in_maps.append(m)
    return in_maps, with_bias


_NC_CACHE = {}


def _get_nc(M, IN_F, O_SH, with_bias):
    key = (M, IN_F, O_SH, with_bias)
    if key not in _NC_CACHE:
        _NC_CACHE[key] = build_nc(M, IN_F, O_SH, with_bias)
    return _NC_CACHE[key]


def kernel(x, weight, bias, _trace=False):
    from concourse.bass_utils import run_bass_kernel_spmd

    N_CORES = 8
    x = np.asarray(x)
    weight = np.asarray(weight)
    bias = np.asarray(bias)
    IN_F = x.shape[-1]
    OUT_F = weight.shape[0]
    M = int(np.prod(x.shape[:-1]))
    O_SH = OUT_F // N_CORES

    in_maps, with_bias = host_prep(x, weight, bias, N_CORES)
    nc = _get_nc(M, IN_F, O_SH, with_bias)
    res = run_bass_kernel_spmd(
        nc, in_maps, core_ids=list(range(N_CORES)), trace=_trace
    )
    parts = [res.results[c]["out"].reshape(*x.shape[:-1], O_SH) for c in range(N_CORES)]
    full = np.concatenate(parts, axis=-1)
    if with_bias is False and np.any(bias):  # pragma: no cover (safety)
        full = full + bias
    if _trace:
        return full, res
    return full
